# revision 38
# baseline (speedup 1.0000x reference)
"""FCAM (DANet-style channel attention, two branches) on 8 Trainium2 cores.

Sharding: pure data-parallel over batch B=32 -> 4 samples/core; all weights
replicated.  Per sample and branch (C=256, spatial N, M=N/8):

  q = qw @ x, k = kw @ x          (1x1 convs, computed directly transposed)
  E = q_r @ k_r^T                 (q_r = q viewed as [256, M])
  attn = softmax(rowmax(E) - E)   == exp(rowmin - E) / rowsum
  out  = alpha * attn @ (vw @ z) + z
       = (attn @ (alpha*vw)) @ z + z     <- reassociated: kills the big v conv

Device-side per core:
  stage A: x streamed in 128-col chunks; one matmul per (chunk, c-half) yields
           qT/kT in PSUM; a strided copy scatters them into the quirky
           [256, M] -> [m-part, c'] interleaved layout (c' = o*8 + p).
           E accumulated over m-chunks; softmax row stats on DVE/ACT with the
           exp+rowsum fused in one activation; attn normalized, cast bf16,
           transposed on the PE; W2T = vw^T-side product so the final matmul
           has its contraction on the partition dim.
  stage B: out_psum[c,n] = sum_e W2T[e,c]*z[e,n] + I@z  (residual folded into
           the accumulation group as an identity matmul), evacuated fp32.

All matmul inputs bf16 (fp32 PSUM accumulation); measured end-to-end
rel-L2 error vs the fp32 reference ~2.4e-3.
"""

import sys

if "/opt/trn_rl_repo" not in sys.path:
    sys.path.insert(0, "/opt/trn_rl_repo")

import numpy as np
import ml_dtypes

BF16 = ml_dtypes.bfloat16

B, C, CQ = 32, 256, 32
N_CORES = 8
BPC = B // N_CORES  # samples per core

# branch name -> (spatial N, m-chunks J); M = N/8 = J*128
BRANCHES = {"t": (1024, 1), "s": (4096, 4)}

_CACHE = {}


def _build(outer_reps=1):
    import concourse.bacc as bacc
    import concourse.tile as tile
    from concourse import mybir

    f32 = mybir.dt.float32
    f16 = mybir.dt.float16
    bf16 = mybir.dt.bfloat16

    nc = bacc.Bacc(
        "TRN2",
        target_bir_lowering=False,
        debug=False,
        enable_asserts=False,
        num_devices=N_CORES,
    )

    dram = {}
    for br, (N, J) in BRANCHES.items():
        dram[f"x_{br}"] = nc.dram_tensor(f"x_{br}", [BPC, C, N], bf16, kind="ExternalInput").ap()
        dram[f"z_{br}"] = nc.dram_tensor(f"z_{br}", [BPC, C, N], bf16, kind="ExternalInput").ap()
        # fp16 output (host upcasts to fp32): ~5e-4 rounding, halves store traffic
        dram[f"out_{br}"] = nc.dram_tensor(f"out_{br}", [BPC, C, N], f16, kind="ExternalOutput").ap()
    # all replicated constants in one pre-laid-out [128, 1408] block (one DMA):
    # [ident 128 | qkT_t 128 | qkT_s 128 | vw_t 512 | vw_s 512] per partition
    dram["consts"] = nc.dram_tensor("consts", [128, 1408], bf16, kind="ExternalInput").ap()

    with tile.TileContext(nc) as tc:
        with (
            tc.tile_pool(name="const", bufs=1) as constp,
            tc.tile_pool(name="io", bufs=2) as iop,
            tc.tile_pool(name="work", bufs=2) as workp,
            tc.tile_pool(name="stat", bufs=3) as statp,
            tc.tile_pool(name="outp", bufs=4) as outp,
            tc.tile_pool(name="psA", bufs=4, space="PSUM") as psA,
            tc.tile_pool(name="psB", bufs=4, space="PSUM") as psB,
        ):
            # ---- replicated constants (single DMA) ----
            const_sb = constp.tile([128, 1408], bf16)
            nc.sync.dma_start(const_sb[:], dram["consts"][:])
            ident_sb = const_sb[:, 0:128]
            qkT_sb = {
                br: const_sb[:, 128 + i * 128:128 + (i + 1) * 128].rearrange(
                    "m (a w) -> m a w", w=2 * CQ
                )
                for i, br in enumerate(BRANCHES)
            }
            vw_sb = {
                br: const_sb[:, 384 + i * 512:384 + (i + 1) * 512].rearrange(
                    "m (a e) -> m a e", e=C
                )
                for i, br in enumerate(BRANCHES)
            }

            # qT/kT columns are kept in the 'sigma' order they fall out of the
            # conv matmuls (col g = p*32+o <-> channel o*8+p); E/attn live in
            # that permuted basis.  The AT evacuation unscrambles the free dim
            # and the host permutes vw's rows, so everything downstream is in
            # natural channel order.
            #
            # Emission is software-pipelined: stage-B tiles of sample s-1 are
            # emitted between stage-A phases of sample s, so the (in-order) PE
            # always has ready matmuls and the HAM clock stays warm.

            def a0_prefetch_all(s):
                """Allocate + DMA x/z one sample ahead of use.  All x issues
                precede all z issues: a z slot-wait on the Sync queue must not
                head-of-line-block the x loads that gate the next qk matmuls."""
                states = {}
                for br, (N, J) in BRANCHES.items():
                    x_sb = iop.tile([128, 2, N], bf16, name=f"x_{br}", tag=f"x_{br}", bufs=3)
                    z_sb = iop.tile([128, 2, N], bf16, name=f"z_{br}", tag=f"z_{br}", bufs=3)
                    states[br] = dict(x_sb=x_sb, z_sb=z_sb)
                for br in BRANCHES:
                    for a in range(2):
                        nc.sync.dma_start(states[br]["x_sb"][:, a, :], dram[f"x_{br}"][s, a * 128:(a + 1) * 128, :])
                for br in BRANCHES:
                    for a in range(2):
                        nc.sync.dma_start(states[br]["z_sb"][:, a, :], dram[f"z_{br}"][s, a * 128:(a + 1) * 128, :])
                return states

            def a1_qk(st, br):
                """q/k conv matmuls + contiguous sigma-order evacuation."""
                N, J = BRANCHES[br]
                x_sb, z_sb = st["x_sb"], st["z_sb"]

                qkt_sb = workp.tile([128, J, 2, C], bf16, name=f"qkt_{br}")
                for j in range(J):
                    qk_ps = psA.tile([128, 8, 2 * CQ], f32, name="qk_ps", tag="ps")
                    for p in range(8):
                        h = p * J + j  # 128-col chunk of x
                        for a in range(2):
                            nc.tensor.matmul(
                                qk_ps[:, p, :],
                                lhsT=x_sb[:, a, h * 128:(h + 1) * 128],
                                rhs=qkT_sb[br][:, a, :],
                                start=(a == 0),
                                stop=(a == 1),
                            )
                    # one copy splits q|k halves into [j, 0, :] / [j, 1, :]
                    nc.scalar.copy(
                        qkt_sb[:, j].rearrange("m h (p o) -> m p h o", o=CQ),
                        qk_ps[:].rearrange("m p (h o) -> m p h o", o=CQ),
                    )
                st["qkt_sb"] = qkt_sb

            def a2_energy_softmax(st, br):
                """E accumulation + reversed softmax (rows/cols in sigma order)."""
                N, J = BRANCHES[br]
                a_n = workp.tile([128, 2, C], bf16, name="a_n", tag=f"a_n_{br}")
                rowmin = statp.tile([128, 2], f32, name="rowmin", tag=f"rowmin_{br}")
                zsum = statp.tile([128, 2], f32, name="zsum", tag=f"zsum_{br}")
                rz = statp.tile([128, 2], f32, name="rz", tag=f"rz_{br}")
                a_raw = workp.tile([128, 2, C], bf16, name="a_raw", tag=f"a_raw_{br}")
                for cc in range(2):
                    e_ps = psA.tile([128, C], f32, name="e_ps", tag="ps")
                    for j in range(J):
                        nc.tensor.matmul(
                            e_ps[:],
                            lhsT=st["qkt_sb"][:, j, 0, cc * 128:(cc + 1) * 128],
                            rhs=st["qkt_sb"][:, j, 1, :],
                            start=(j == 0),
                            stop=(j == J - 1),
                        )
                    nc.vector.tensor_reduce(
                        rowmin[:, cc:cc + 1], e_ps[:],
                        axis=mybir.AxisListType.X, op=mybir.AluOpType.min,
                    )
                    # a_raw = exp(rowmin - E), zsum = row-sum, fused
                    nc.scalar.activation(
                        a_raw[:, cc, :], e_ps[:],
                        mybir.ActivationFunctionType.Exp,
                        bias=rowmin[:, cc:cc + 1], scale=-1.0,
                        accum_out=zsum[:, cc:cc + 1],
                    )
                    nc.vector.reciprocal(rz[:, cc:cc + 1], zsum[:, cc:cc + 1])
                    nc.vector.tensor_scalar_mul(
                        a_n[:, cc, :], a_raw[:, cc, :], rz[:, cc:cc + 1]
                    )
                st["a_n"] = a_n

            def a3_transpose(st, br):
                """attn^T on PE; evacuation unscrambles the free dim to natural order."""
                at_sb = workp.tile([128, 2, C], bf16, name="at_sb", tag=f"at_{br}")
                for dc in range(2):
                    at_ps = psB.tile([128, C], bf16, name="at_ps", tag="ps")
                    for cc in range(2):
                        nc.tensor.transpose(
                            at_ps[:, cc * 128:(cc + 1) * 128],
                            st["a_n"][:, cc, dc * 128:(dc + 1) * 128],
                            ident_sb[:],
                        )
                    dst = at_sb[:, dc, :].rearrange("m (o k) -> m k o", k=8)
                    src = at_ps[:].rearrange("m (p o) -> m p o", o=CQ)
                    if dc == 0:
                        nc.vector.tensor_copy(dst, src)
                    else:
                        nc.scalar.copy(dst, src)
                st["at_sb"] = at_sb

            def a4_w2t(st, br):
                """W2T[e, c] = sum_d' vw_sigma[d', e] * attnT[d', c]."""
                w2t_sb = workp.tile([128, 2, C], bf16, name="w2t_sb", tag=f"w2t_{br}")
                for ec in range(2):
                    w2t_ps = psB.tile([128, C], f32, name="w2t_ps", tag="ps")
                    for dc in range(2):
                        nc.tensor.matmul(
                            w2t_ps[:],
                            lhsT=vw_sb[br][:, dc, ec * 128:(ec + 1) * 128],
                            rhs=st["at_sb"][:, dc, :],
                            start=(dc == 0),
                            stop=(dc == 1),
                        )
                    nc.scalar.copy(w2t_sb[:, ec, :], w2t_ps[:])
                st["w2t_sb"] = w2t_sb

            def b_thunks(s, states):
                """One thunk per output tile: out = W2 @ z + z on a 512-col slice."""
                thunks = []
                for br in BRANCHES:
                    N, _ = BRANCHES[br]
                    z_sb, w2t_sb = states[br]["z_sb"], states[br]["w2t_sb"]

                    def tile_thunk(br=br, s=s, z_sb=z_sb, w2t_sb=w2t_sb, ns=0, cc=0):
                        o_ps = psB.tile([128, 512], f32, name="o_ps", tag="ps")
                        for ec in range(2):
                            nc.tensor.matmul(
                                o_ps[:],
                                lhsT=w2t_sb[:, ec, cc * 128:(cc + 1) * 128],
                                rhs=z_sb[:, ec, ns * 512:(ns + 1) * 512],
                                start=(ec == 0),
                                stop=False,
                            )
                        nc.tensor.matmul(
                            o_ps[:],
                            lhsT=ident_sb[:],
                            rhs=z_sb[:, cc, ns * 512:(ns + 1) * 512],
                            start=False,
                            stop=True,
                        )
                        o_sb = outp.tile([128, 512], f16, name="o_sb")
                        # all B evacuations on DVE: keeps ScalarE clear for the
                        # critical-path softmax exp
                        nc.vector.tensor_copy(o_sb[:], o_ps[:])
                        nc.sync.dma_start(
                            dram[f"out_{br}"][s, cc * 128:(cc + 1) * 128, ns * 512:(ns + 1) * 512],
                            o_sb[:],
                        )

                    for ns in range(N // 512):
                        for cc in range(2):
                            thunks.append(
                                lambda f=tile_thunk, ns=ns, cc=cc: f(ns=ns, cc=cc)
                            )
                return thunks

            pending = []

            def drain(k):
                for _ in range(min(k, len(pending))):
                    pending.pop(0)()

            samples = [s for _ in range(outer_reps) for s in range(BPC)]
            prefetched = a0_prefetch_all(samples[0])
            for si, s in enumerate(samples):
                states = prefetched
                if si + 1 < len(samples):
                    prefetched = a0_prefetch_all(samples[si + 1])
                for br in BRANCHES:
                    a1_qk(states[br], br)
                drain(6)
                for br in BRANCHES:
                    a2_energy_softmax(states[br], br)
                drain(6)
                for br in BRANCHES:
                    a3_transpose(states[br], br)
                for br in BRANCHES:
                    a4_w2t(states[br], br)
                drain(len(pending))
                pending = b_thunks(s, states)
            drain(len(pending))

    nc.compile()
    return nc


def _numpy_fallback(inp):
    # Exact fp32 mirror of the reference; only used if conv biases are
    # nonzero (setup_inputs always produces zero biases).
    def conv(x, w, b):
        return np.einsum("bchw,oc->bohw", x, w) + b[None, :, None, None]

    def branch(rgb, z, qw, qb, kw, kb, vw, vb):
        Bb, Cc, H, W = rgb.shape
        q = conv(rgb, qw, qb).reshape(Bb, Cc, -1)
        k = conv(rgb, kw, kb).reshape(Bb, Cc, -1)
        e = np.einsum("bcm,bdm->bcd", q, k)
        e = e.max(-1, keepdims=True) - e
        e = e - e.max(-1, keepdims=True)
        a = np.exp(e)
        a /= a.sum(-1, keepdims=True)
        v = conv(z, vw, vb).reshape(Bb, Cc, -1)
        return np.einsum("bcd,bdn->bcn", a, v).reshape(Bb, Cc, H, W)

    t = inp["alpha"] * branch(inp["rgb_t_map"], inp["z_t_map"], inp["tq_w"], inp["tq_b"],
                              inp["tk_w"], inp["tk_b"], inp["tv_w"], inp["tv_b"]) + inp["z_t_map"]
    s = inp["beta"] * branch(inp["rgb_s_map"], inp["z_s_map"], inp["sq_w"], inp["sq_b"],
                             inp["sk_w"], inp["sk_b"], inp["sv_w"], inp["sv_b"]) + inp["z_s_map"]
    return (t.astype(np.float32), s.astype(np.float32))


def _run(in_maps, trace=False):
    from concourse import bass_utils

    if "nc" not in _CACHE:
        _CACHE["nc"] = _build()
    return bass_utils.run_bass_kernel_spmd(
        _CACHE["nc"], in_maps, core_ids=list(range(N_CORES)), trace=trace
    )


def _prep_in_maps(inputs):
    inp = {k: np.asarray(v) for k, v in inputs.items()}
    x_t = inp["rgb_t_map"].reshape(B, C, -1).astype(BF16)
    z_t = inp["z_t_map"].reshape(B, C, -1).astype(BF16)
    x_s = inp["rgb_s_map"].reshape(B, C, -1).astype(BF16)
    z_s = inp["z_s_map"].reshape(B, C, -1).astype(BF16)
    # attn rows/cols live in the sigma-permuted basis (col g = p*32+o holds
    # channel o*8+p), so vw's contraction rows get the same permutation.
    g = np.arange(C)
    sigma = (g % 32) * 8 + g // 32

    def split(a):  # [256, W] -> [128, 2*W] (partition p holds rows p and 128+p)
        return a.reshape(2, 128, -1).transpose(1, 0, 2).reshape(128, -1)

    consts = np.concatenate(
        [
            np.eye(128, dtype=np.float32),
            split(np.concatenate([inp["tq_w"].T, inp["tk_w"].T], axis=1)),
            split(np.concatenate([inp["sq_w"].T, inp["sk_w"].T], axis=1)),
            split((float(inp["alpha"][0]) * inp["tv_w"])[sigma, :]),
            split((float(inp["beta"][0]) * inp["sv_w"])[sigma, :]),
        ],
        axis=1,
    )
    rep = {"consts": np.ascontiguousarray(consts).astype(BF16)}
    in_maps = []
    for i in range(N_CORES):
        sl = slice(i * BPC, (i + 1) * BPC)
        in_maps.append({
            "x_t": np.ascontiguousarray(x_t[sl]),
            "z_t": np.ascontiguousarray(z_t[sl]),
            "x_s": np.ascontiguousarray(x_s[sl]),
            "z_s": np.ascontiguousarray(z_s[sl]),
            **rep,
        })
    return in_maps


def kernel(**inputs):
    inp = {k: np.asarray(v) for k, v in inputs.items()}
    if any(np.any(inp[k]) for k in ("tq_b", "tk_b", "tv_b", "sq_b", "sk_b", "sv_b")):
        return _numpy_fallback(inp)

    res = _run(_prep_in_maps(inp))
    out_t = np.concatenate([r["out_t"] for r in res.results]).reshape(B, C, 32, 32)
    out_s = np.concatenate([r["out_s"] for r in res.results]).reshape(B, C, 64, 64)
    return (out_t.astype(np.float32), out_s.astype(np.float32))


# revision 40
# speedup vs baseline: 1.0023x; 1.0023x over previous
"""FCAM (DANet-style channel attention, two branches) on 8 Trainium2 cores.

Sharding: pure data-parallel over batch B=32 -> 4 samples/core; all weights
replicated.  Per sample and branch (C=256, spatial N, M=N/8):

  q = qw @ x, k = kw @ x          (1x1 convs, computed directly transposed)
  E = q_r @ k_r^T                 (q_r = q viewed as [256, M])
  attn = softmax(rowmax(E) - E)   == exp(rowmin - E) / rowsum
  out  = alpha * attn @ (vw @ z) + z
       = (attn @ (alpha*vw)) @ z + z     <- reassociated: kills the big v conv

Device-side per core:
  stage A: x streamed in 128-col chunks; one matmul per (chunk, c-half) yields
           qT/kT in PSUM; a strided copy scatters them into the quirky
           [256, M] -> [m-part, c'] interleaved layout (c' = o*8 + p).
           E accumulated over m-chunks; softmax row stats on DVE/ACT with the
           exp+rowsum fused in one activation; attn normalized, cast bf16,
           transposed on the PE; W2T = vw^T-side product so the final matmul
           has its contraction on the partition dim.
  stage B: out_psum[c,n] = sum_e W2T[e,c]*z[e,n] + I@z  (residual folded into
           the accumulation group as an identity matmul), evacuated fp32.

All matmul inputs bf16 (fp32 PSUM accumulation); measured end-to-end
rel-L2 error vs the fp32 reference ~2.4e-3.
"""

import sys

if "/opt/trn_rl_repo" not in sys.path:
    sys.path.insert(0, "/opt/trn_rl_repo")

import numpy as np
import ml_dtypes

BF16 = ml_dtypes.bfloat16

B, C, CQ = 32, 256, 32
N_CORES = 8
BPC = B // N_CORES  # samples per core

# branch name -> (spatial N, m-chunks J); M = N/8 = J*128
BRANCHES = {"t": (1024, 1), "s": (4096, 4)}

_CACHE = {}


def _build(outer_reps=1):
    import concourse.bacc as bacc
    import concourse.tile as tile
    from concourse import mybir

    f32 = mybir.dt.float32
    f16 = mybir.dt.float16
    bf16 = mybir.dt.bfloat16

    nc = bacc.Bacc(
        "TRN2",
        target_bir_lowering=False,
        debug=False,
        enable_asserts=False,
        num_devices=N_CORES,
    )

    dram = {}
    for br, (N, J) in BRANCHES.items():
        dram[f"x_{br}"] = nc.dram_tensor(f"x_{br}", [BPC, C, N], bf16, kind="ExternalInput").ap()
        dram[f"z_{br}"] = nc.dram_tensor(f"z_{br}", [BPC, C, N], bf16, kind="ExternalInput").ap()
        # fp16 output (host upcasts to fp32): ~5e-4 rounding, halves store traffic
        dram[f"out_{br}"] = nc.dram_tensor(f"out_{br}", [BPC, C, N], f16, kind="ExternalOutput").ap()
    # all replicated constants in one pre-laid-out [128, 1408] block (one DMA):
    # [ident 128 | qkT_t 128 | qkT_s 128 | vw_t 512 | vw_s 512] per partition
    dram["consts"] = nc.dram_tensor("consts", [128, 1408], bf16, kind="ExternalInput").ap()

    with tile.TileContext(nc) as tc:
        with (
            tc.tile_pool(name="const", bufs=1) as constp,
            tc.tile_pool(name="io", bufs=2) as iop,
            tc.tile_pool(name="work", bufs=2) as workp,
            tc.tile_pool(name="stat", bufs=3) as statp,
            tc.tile_pool(name="outp", bufs=4) as outp,
            tc.tile_pool(name="psA", bufs=4, space="PSUM") as psA,
            tc.tile_pool(name="psB", bufs=4, space="PSUM") as psB,
        ):
            # ---- replicated constants (single DMA) ----
            const_sb = constp.tile([128, 1408], bf16)
            nc.sync.dma_start(const_sb[:], dram["consts"][:])
            ident_sb = const_sb[:, 0:128]
            qkT_sb = {
                br: const_sb[:, 128 + i * 128:128 + (i + 1) * 128].rearrange(
                    "m (a w) -> m a w", w=2 * CQ
                )
                for i, br in enumerate(BRANCHES)
            }
            vw_sb = {
                br: const_sb[:, 384 + i * 512:384 + (i + 1) * 512].rearrange(
                    "m (a e) -> m a e", e=C
                )
                for i, br in enumerate(BRANCHES)
            }

            # qT/kT columns are kept in the 'sigma' order they fall out of the
            # conv matmuls (col g = p*32+o <-> channel o*8+p); E/attn live in
            # that permuted basis.  The AT evacuation unscrambles the free dim
            # and the host permutes vw's rows, so everything downstream is in
            # natural channel order.
            #
            # Emission is software-pipelined: stage-B tiles of sample s-1 are
            # emitted between stage-A phases of sample s, so the (in-order) PE
            # always has ready matmuls and the HAM clock stays warm.

            def a0_prefetch_all(s):
                """Allocate + DMA x/z one sample ahead of use.  All x issues
                precede all z issues: a z slot-wait on the Sync queue must not
                head-of-line-block the x loads that gate the next qk matmuls."""
                states = {}
                for br, (N, J) in BRANCHES.items():
                    x_sb = iop.tile([128, 2, N], bf16, name=f"x_{br}", tag=f"x_{br}", bufs=3)
                    z_sb = iop.tile([128, 2, N], bf16, name=f"z_{br}", tag=f"z_{br}", bufs=3)
                    states[br] = dict(x_sb=x_sb, z_sb=z_sb)
                    for a in range(2):
                        nc.sync.dma_start(x_sb[:, a, :], dram[f"x_{br}"][s, a * 128:(a + 1) * 128, :])
                        nc.sync.dma_start(z_sb[:, a, :], dram[f"z_{br}"][s, a * 128:(a + 1) * 128, :])
                return states

            def a1_qk(st, br):
                """q/k conv matmuls + contiguous sigma-order evacuation."""
                N, J = BRANCHES[br]
                x_sb, z_sb = st["x_sb"], st["z_sb"]

                qkt_sb = workp.tile([128, J, 2, C], bf16, name=f"qkt_{br}")
                for j in range(J):
                    qk_ps = psA.tile([128, 8, 2 * CQ], f32, name="qk_ps", tag="ps")
                    for p in range(8):
                        h = p * J + j  # 128-col chunk of x
                        for a in range(2):
                            nc.tensor.matmul(
                                qk_ps[:, p, :],
                                lhsT=x_sb[:, a, h * 128:(h + 1) * 128],
                                rhs=qkT_sb[br][:, a, :],
                                start=(a == 0),
                                stop=(a == 1),
                            )
                    # one copy splits q|k halves into [j, 0, :] / [j, 1, :]
                    nc.scalar.copy(
                        qkt_sb[:, j].rearrange("m h (p o) -> m p h o", o=CQ),
                        qk_ps[:].rearrange("m p (h o) -> m p h o", o=CQ),
                    )
                st["qkt_sb"] = qkt_sb

            def a2_energy_softmax(st, br):
                """E accumulation + reversed softmax (rows/cols in sigma order)."""
                N, J = BRANCHES[br]
                a_n = workp.tile([128, 2, C], bf16, name="a_n", tag=f"a_n_{br}")
                rowmin = statp.tile([128, 2], f32, name="rowmin", tag=f"rowmin_{br}")
                zsum = statp.tile([128, 2], f32, name="zsum", tag=f"zsum_{br}")
                rz = statp.tile([128, 2], f32, name="rz", tag=f"rz_{br}")
                a_raw = workp.tile([128, 2, C], bf16, name="a_raw", tag=f"a_raw_{br}")
                for cc in range(2):
                    e_ps = psA.tile([128, C], f32, name="e_ps", tag="ps")
                    for j in range(J):
                        nc.tensor.matmul(
                            e_ps[:],
                            lhsT=st["qkt_sb"][:, j, 0, cc * 128:(cc + 1) * 128],
                            rhs=st["qkt_sb"][:, j, 1, :],
                            start=(j == 0),
                            stop=(j == J - 1),
                        )
                    nc.vector.tensor_reduce(
                        rowmin[:, cc:cc + 1], e_ps[:],
                        axis=mybir.AxisListType.X, op=mybir.AluOpType.min,
                    )
                    # a_raw = exp(rowmin - E), zsum = row-sum, fused
                    nc.scalar.activation(
                        a_raw[:, cc, :], e_ps[:],
                        mybir.ActivationFunctionType.Exp,
                        bias=rowmin[:, cc:cc + 1], scale=-1.0,
                        accum_out=zsum[:, cc:cc + 1],
                    )
                    nc.vector.reciprocal(rz[:, cc:cc + 1], zsum[:, cc:cc + 1])
                    nc.vector.tensor_scalar_mul(
                        a_n[:, cc, :], a_raw[:, cc, :], rz[:, cc:cc + 1]
                    )
                st["a_n"] = a_n

            def a3_transpose(st, br):
                """attn^T on PE; evacuation unscrambles the free dim to natural order."""
                at_sb = workp.tile([128, 2, C], bf16, name="at_sb", tag=f"at_{br}")
                for dc in range(2):
                    at_ps = psB.tile([128, C], bf16, name="at_ps", tag="ps")
                    for cc in range(2):
                        nc.tensor.transpose(
                            at_ps[:, cc * 128:(cc + 1) * 128],
                            st["a_n"][:, cc, dc * 128:(dc + 1) * 128],
                            ident_sb[:],
                        )
                    dst = at_sb[:, dc, :].rearrange("m (o k) -> m k o", k=8)
                    src = at_ps[:].rearrange("m (p o) -> m p o", o=CQ)
                    if dc == 0:
                        nc.vector.tensor_copy(dst, src)
                    else:
                        nc.scalar.copy(dst, src)
                st["at_sb"] = at_sb

            def a4_w2t(st, br):
                """W2T[e, c] = sum_d' vw_sigma[d', e] * attnT[d', c]."""
                w2t_sb = workp.tile([128, 2, C], bf16, name="w2t_sb", tag=f"w2t_{br}")
                for ec in range(2):
                    w2t_ps = psB.tile([128, C], f32, name="w2t_ps", tag="ps")
                    for dc in range(2):
                        nc.tensor.matmul(
                            w2t_ps[:],
                            lhsT=vw_sb[br][:, dc, ec * 128:(ec + 1) * 128],
                            rhs=st["at_sb"][:, dc, :],
                            start=(dc == 0),
                            stop=(dc == 1),
                        )
                    nc.scalar.copy(w2t_sb[:, ec, :], w2t_ps[:])
                st["w2t_sb"] = w2t_sb

            def b_thunks(s, states):
                """One thunk per output tile: out = W2 @ z + z on a 512-col slice."""
                thunks = []
                for br in BRANCHES:
                    N, _ = BRANCHES[br]
                    z_sb, w2t_sb = states[br]["z_sb"], states[br]["w2t_sb"]

                    def tile_thunk(br=br, s=s, z_sb=z_sb, w2t_sb=w2t_sb, ns=0, cc=0):
                        o_ps = psB.tile([128, 512], f32, name="o_ps", tag="ps")
                        for ec in range(2):
                            nc.tensor.matmul(
                                o_ps[:],
                                lhsT=w2t_sb[:, ec, cc * 128:(cc + 1) * 128],
                                rhs=z_sb[:, ec, ns * 512:(ns + 1) * 512],
                                start=(ec == 0),
                                stop=False,
                            )
                        nc.tensor.matmul(
                            o_ps[:],
                            lhsT=ident_sb[:],
                            rhs=z_sb[:, cc, ns * 512:(ns + 1) * 512],
                            start=False,
                            stop=True,
                        )
                        o_sb = outp.tile([128, 512], f16, name="o_sb")
                        # all B evacuations on DVE: keeps ScalarE clear for the
                        # critical-path softmax exp
                        nc.vector.tensor_copy(o_sb[:], o_ps[:])
                        nc.sync.dma_start(
                            dram[f"out_{br}"][s, cc * 128:(cc + 1) * 128, ns * 512:(ns + 1) * 512],
                            o_sb[:],
                        )

                    for ns in range(N // 512):
                        for cc in range(2):
                            thunks.append(
                                lambda f=tile_thunk, ns=ns, cc=cc: f(ns=ns, cc=cc)
                            )
                return thunks

            pending = []

            def drain(k):
                for _ in range(min(k, len(pending))):
                    pending.pop(0)()

            samples = [s for _ in range(outer_reps) for s in range(BPC)]
            prefetched = a0_prefetch_all(samples[0])
            for si, s in enumerate(samples):
                states = prefetched
                if si + 1 < len(samples):
                    prefetched = a0_prefetch_all(samples[si + 1])
                for br in BRANCHES:
                    a1_qk(states[br], br)
                drain(6)
                for br in BRANCHES:
                    a2_energy_softmax(states[br], br)
                drain(6)
                for br in BRANCHES:
                    a3_transpose(states[br], br)
                for br in BRANCHES:
                    a4_w2t(states[br], br)
                drain(len(pending))
                pending = b_thunks(s, states)
            drain(len(pending))

    nc.compile()
    return nc


def _numpy_fallback(inp):
    # Exact fp32 mirror of the reference; only used if conv biases are
    # nonzero (setup_inputs always produces zero biases).
    def conv(x, w, b):
        return np.einsum("bchw,oc->bohw", x, w) + b[None, :, None, None]

    def branch(rgb, z, qw, qb, kw, kb, vw, vb):
        Bb, Cc, H, W = rgb.shape
        q = conv(rgb, qw, qb).reshape(Bb, Cc, -1)
        k = conv(rgb, kw, kb).reshape(Bb, Cc, -1)
        e = np.einsum("bcm,bdm->bcd", q, k)
        e = e.max(-1, keepdims=True) - e
        e = e - e.max(-1, keepdims=True)
        a = np.exp(e)
        a /= a.sum(-1, keepdims=True)
        v = conv(z, vw, vb).reshape(Bb, Cc, -1)
        return np.einsum("bcd,bdn->bcn", a, v).reshape(Bb, Cc, H, W)

    t = inp["alpha"] * branch(inp["rgb_t_map"], inp["z_t_map"], inp["tq_w"], inp["tq_b"],
                              inp["tk_w"], inp["tk_b"], inp["tv_w"], inp["tv_b"]) + inp["z_t_map"]
    s = inp["beta"] * branch(inp["rgb_s_map"], inp["z_s_map"], inp["sq_w"], inp["sq_b"],
                             inp["sk_w"], inp["sk_b"], inp["sv_w"], inp["sv_b"]) + inp["z_s_map"]
    return (t.astype(np.float32), s.astype(np.float32))


def _run(in_maps, trace=False):
    from concourse import bass_utils

    if "nc" not in _CACHE:
        _CACHE["nc"] = _build()
    return bass_utils.run_bass_kernel_spmd(
        _CACHE["nc"], in_maps, core_ids=list(range(N_CORES)), trace=trace
    )


def _prep_in_maps(inputs):
    inp = {k: np.asarray(v) for k, v in inputs.items()}
    x_t = inp["rgb_t_map"].reshape(B, C, -1).astype(BF16)
    z_t = inp["z_t_map"].reshape(B, C, -1).astype(BF16)
    x_s = inp["rgb_s_map"].reshape(B, C, -1).astype(BF16)
    z_s = inp["z_s_map"].reshape(B, C, -1).astype(BF16)
    # attn rows/cols live in the sigma-permuted basis (col g = p*32+o holds
    # channel o*8+p), so vw's contraction rows get the same permutation.
    g = np.arange(C)
    sigma = (g % 32) * 8 + g // 32

    def split(a):  # [256, W] -> [128, 2*W] (partition p holds rows p and 128+p)
        return a.reshape(2, 128, -1).transpose(1, 0, 2).reshape(128, -1)

    consts = np.concatenate(
        [
            np.eye(128, dtype=np.float32),
            split(np.concatenate([inp["tq_w"].T, inp["tk_w"].T], axis=1)),
            split(np.concatenate([inp["sq_w"].T, inp["sk_w"].T], axis=1)),
            split((float(inp["alpha"][0]) * inp["tv_w"])[sigma, :]),
            split((float(inp["beta"][0]) * inp["sv_w"])[sigma, :]),
        ],
        axis=1,
    )
    rep = {"consts": np.ascontiguousarray(consts).astype(BF16)}
    in_maps = []
    for i in range(N_CORES):
        sl = slice(i * BPC, (i + 1) * BPC)
        in_maps.append({
            "x_t": np.ascontiguousarray(x_t[sl]),
            "z_t": np.ascontiguousarray(z_t[sl]),
            "x_s": np.ascontiguousarray(x_s[sl]),
            "z_s": np.ascontiguousarray(z_s[sl]),
            **rep,
        })
    return in_maps


def kernel(**inputs):
    inp = {k: np.asarray(v) for k, v in inputs.items()}
    if any(np.any(inp[k]) for k in ("tq_b", "tk_b", "tv_b", "sq_b", "sk_b", "sv_b")):
        return _numpy_fallback(inp)

    res = _run(_prep_in_maps(inp))
    out_t = np.concatenate([r["out_t"] for r in res.results]).reshape(B, C, 32, 32)
    out_s = np.concatenate([r["out_s"] for r in res.results]).reshape(B, C, 64, 64)
    return (out_t.astype(np.float32), out_s.astype(np.float32))


# revision 41
# speedup vs baseline: 1.0554x; 1.0529x over previous
"""FCAM (DANet-style channel attention, two branches) on 8 Trainium2 cores.

Sharding: pure data-parallel over batch B=32 -> 4 samples/core; all weights
replicated.  Per sample and branch (C=256, spatial N, M=N/8):

  q = qw @ x, k = kw @ x          (1x1 convs, computed directly transposed)
  E = q_r @ k_r^T                 (q_r = q viewed as [256, M])
  attn = softmax(rowmax(E) - E)   == exp(rowmin - E) / rowsum
  out  = alpha * attn @ (vw @ z) + z
       = (attn @ (alpha*vw)) @ z + z     <- reassociated: kills the big v conv

Device-side per core:
  stage A: x streamed in 128-col chunks; one matmul per (chunk, c-half) yields
           qT/kT in PSUM; a strided copy scatters them into the quirky
           [256, M] -> [m-part, c'] interleaved layout (c' = o*8 + p).
           E accumulated over m-chunks; softmax row stats on DVE/ACT with the
           exp+rowsum fused in one activation; attn normalized, cast bf16,
           transposed on the PE; W2T = vw^T-side product so the final matmul
           has its contraction on the partition dim.
  stage B: out_psum[c,n] = sum_e W2T[e,c]*z[e,n] + I@z  (residual folded into
           the accumulation group as an identity matmul), evacuated fp32.

All matmul inputs bf16 (fp32 PSUM accumulation); measured end-to-end
rel-L2 error vs the fp32 reference ~2.4e-3.
"""

import sys

if "/opt/trn_rl_repo" not in sys.path:
    sys.path.insert(0, "/opt/trn_rl_repo")

import numpy as np
import ml_dtypes

BF16 = ml_dtypes.bfloat16

B, C, CQ = 32, 256, 32
N_CORES = 8
BPC = B // N_CORES  # samples per core

# branch name -> (spatial N, m-chunks J); M = N/8 = J*128
BRANCHES = {"t": (1024, 1), "s": (4096, 4)}

_CACHE = {}


def _build(outer_reps=1):
    import concourse.bacc as bacc
    import concourse.tile as tile
    from concourse import mybir

    f32 = mybir.dt.float32
    f16 = mybir.dt.float16
    bf16 = mybir.dt.bfloat16

    nc = bacc.Bacc(
        "TRN2",
        target_bir_lowering=False,
        debug=False,
        enable_asserts=False,
        num_devices=N_CORES,
    )

    dram = {}
    for br, (N, J) in BRANCHES.items():
        dram[f"x_{br}"] = nc.dram_tensor(f"x_{br}", [BPC, C, N], bf16, kind="ExternalInput").ap()
        dram[f"z_{br}"] = nc.dram_tensor(f"z_{br}", [BPC, C, N], bf16, kind="ExternalInput").ap()
        # fp16 output (host upcasts to fp32): ~5e-4 rounding, halves store traffic
        dram[f"out_{br}"] = nc.dram_tensor(f"out_{br}", [BPC, C, N], f16, kind="ExternalOutput").ap()
    # all replicated constants in one pre-laid-out [128, 1408] block (one DMA):
    # [ident 128 | qkT_t 128 | qkT_s 128 | vw_t 512 | vw_s 512] per partition
    dram["consts"] = nc.dram_tensor("consts", [128, 1408], bf16, kind="ExternalInput").ap()

    with tile.TileContext(nc) as tc:
        with (
            tc.tile_pool(name="const", bufs=1) as constp,
            tc.tile_pool(name="io", bufs=2) as iop,
            tc.tile_pool(name="work", bufs=2) as workp,
            tc.tile_pool(name="stat", bufs=3) as statp,
            tc.tile_pool(name="outp", bufs=4) as outp,
            tc.tile_pool(name="psA", bufs=4, space="PSUM") as psA,
            tc.tile_pool(name="psB", bufs=4, space="PSUM") as psB,
        ):
            # ---- replicated constants (single DMA) ----
            const_sb = constp.tile([128, 1408], bf16)
            nc.sync.dma_start(const_sb[:], dram["consts"][:])
            ident_sb = const_sb[:, 0:128]
            qkT_sb = {
                br: const_sb[:, 128 + i * 128:128 + (i + 1) * 128].rearrange(
                    "m (a w) -> m a w", w=2 * CQ
                )
                for i, br in enumerate(BRANCHES)
            }
            vw_sb = {
                br: const_sb[:, 384 + i * 512:384 + (i + 1) * 512].rearrange(
                    "m (a e) -> m a e", e=C
                )
                for i, br in enumerate(BRANCHES)
            }

            # qT/kT columns are kept in the 'sigma' order they fall out of the
            # conv matmuls (col g = p*32+o <-> channel o*8+p); E/attn live in
            # that permuted basis.  The AT evacuation unscrambles the free dim
            # and the host permutes vw's rows, so everything downstream is in
            # natural channel order.
            #
            # Emission is software-pipelined: stage-B tiles of sample s-1 are
            # emitted between stage-A phases of sample s, so the (in-order) PE
            # always has ready matmuls and the HAM clock stays warm.

            def a0_prefetch_all(s):
                """Allocate + DMA x/z one sample ahead of use.  All x issues
                precede all z issues: a z slot-wait on the Sync queue must not
                head-of-line-block the x loads that gate the next qk matmuls."""
                states = {}
                for br, (N, J) in BRANCHES.items():
                    x_sb = iop.tile([128, 2, N], bf16, name=f"x_{br}", tag=f"x_{br}", bufs=3)
                    z_sb = iop.tile([128, 2, N], bf16, name=f"z_{br}", tag=f"z_{br}", bufs=3)
                    states[br] = dict(x_sb=x_sb, z_sb=z_sb)
                    for a in range(2):
                        nc.sync.dma_start(x_sb[:, a, :], dram[f"x_{br}"][s, a * 128:(a + 1) * 128, :])
                        nc.sync.dma_start(z_sb[:, a, :], dram[f"z_{br}"][s, a * 128:(a + 1) * 128, :])
                return states

            def a1_qk(st, br):
                """q/k conv matmuls + contiguous sigma-order evacuation."""
                N, J = BRANCHES[br]
                x_sb, z_sb = st["x_sb"], st["z_sb"]

                qkt_sb = workp.tile([128, J, 2, C], bf16, name=f"qkt_{br}")
                for j in range(J):
                    qk_ps = psA.tile([128, 8, 2 * CQ], f32, name="qk_ps", tag="ps")
                    for p in range(8):
                        h = p * J + j  # 128-col chunk of x
                        for a in range(2):
                            nc.tensor.matmul(
                                qk_ps[:, p, :],
                                lhsT=x_sb[:, a, h * 128:(h + 1) * 128],
                                rhs=qkT_sb[br][:, a, :],
                                start=(a == 0),
                                stop=(a == 1),
                            )
                    # one copy splits q|k halves into [j, 0, :] / [j, 1, :]
                    nc.scalar.copy(
                        qkt_sb[:, j].rearrange("m h (p o) -> m p h o", o=CQ),
                        qk_ps[:].rearrange("m p (h o) -> m p h o", o=CQ),
                    )
                st["qkt_sb"] = qkt_sb

            def a2_energy_softmax(st, br):
                """E accumulation + reversed softmax (rows/cols in sigma order)."""
                N, J = BRANCHES[br]
                a_n = workp.tile([128, 2, C], bf16, name="a_n", tag=f"a_n_{br}")
                rowmin = statp.tile([128, 2], f32, name="rowmin", tag=f"rowmin_{br}")
                zsum = statp.tile([128, 2], f32, name="zsum", tag=f"zsum_{br}")
                rz = statp.tile([128, 2], f32, name="rz", tag=f"rz_{br}")
                a_raw = workp.tile([128, 2, C], bf16, name="a_raw", tag=f"a_raw_{br}")
                for cc in range(2):
                    e_ps = psA.tile([128, C], f32, name="e_ps", tag="ps")
                    for j in range(J):
                        nc.tensor.matmul(
                            e_ps[:],
                            lhsT=st["qkt_sb"][:, j, 0, cc * 128:(cc + 1) * 128],
                            rhs=st["qkt_sb"][:, j, 1, :],
                            start=(j == 0),
                            stop=(j == J - 1),
                        )
                    nc.vector.tensor_reduce(
                        rowmin[:, cc:cc + 1], e_ps[:],
                        axis=mybir.AxisListType.X, op=mybir.AluOpType.min,
                    )
                    # a_raw = exp(rowmin - E), zsum = row-sum, fused
                    nc.scalar.activation(
                        a_raw[:, cc, :], e_ps[:],
                        mybir.ActivationFunctionType.Exp,
                        bias=rowmin[:, cc:cc + 1], scale=-1.0,
                        accum_out=zsum[:, cc:cc + 1],
                    )
                    nc.vector.reciprocal(rz[:, cc:cc + 1], zsum[:, cc:cc + 1])
                    nc.vector.tensor_scalar_mul(
                        a_n[:, cc, :], a_raw[:, cc, :], rz[:, cc:cc + 1]
                    )
                st["a_n"] = a_n

            def a3_transpose(st, br):
                """attn^T on PE; evacuation unscrambles the free dim to natural order."""
                at_sb = workp.tile([128, 2, C], bf16, name="at_sb", tag=f"at_{br}")
                for dc in range(2):
                    at_ps = psB.tile([128, C], bf16, name="at_ps", tag="ps")
                    for cc in range(2):
                        nc.tensor.transpose(
                            at_ps[:, cc * 128:(cc + 1) * 128],
                            st["a_n"][:, cc, dc * 128:(dc + 1) * 128],
                            ident_sb[:],
                        )
                    dst = at_sb[:, dc, :].rearrange("m (o k) -> m k o", k=8)
                    src = at_ps[:].rearrange("m (p o) -> m p o", o=CQ)
                    if dc == 0:
                        nc.vector.tensor_copy(dst, src)
                    else:
                        nc.scalar.copy(dst, src)
                st["at_sb"] = at_sb

            def a4_w2t(st, br):
                """W2T[e, c] = sum_d' vw_sigma[d', e] * attnT[d', c]."""
                w2t_sb = workp.tile([128, 2, C], bf16, name="w2t_sb", tag=f"w2t_{br}")
                for ec in range(2):
                    w2t_ps = psB.tile([128, C], f32, name="w2t_ps", tag="ps")
                    for dc in range(2):
                        nc.tensor.matmul(
                            w2t_ps[:],
                            lhsT=vw_sb[br][:, dc, ec * 128:(ec + 1) * 128],
                            rhs=st["at_sb"][:, dc, :],
                            start=(dc == 0),
                            stop=(dc == 1),
                        )
                    nc.scalar.copy(w2t_sb[:, ec, :], w2t_ps[:])
                st["w2t_sb"] = w2t_sb

            def b_thunks(s, states):
                """One thunk per output tile: out = W2 @ z + z on a 512-col slice."""
                thunks = []
                for br in BRANCHES:
                    N, _ = BRANCHES[br]
                    z_sb, w2t_sb = states[br]["z_sb"], states[br]["w2t_sb"]

                    def tile_thunk(br=br, s=s, z_sb=z_sb, w2t_sb=w2t_sb, ns=0, cc=0):
                        o_ps = psB.tile([128, 512], f32, name="o_ps", tag="ps")
                        for ec in range(2):
                            nc.tensor.matmul(
                                o_ps[:],
                                lhsT=w2t_sb[:, ec, cc * 128:(cc + 1) * 128],
                                rhs=z_sb[:, ec, ns * 512:(ns + 1) * 512],
                                start=(ec == 0),
                                stop=(ec == 1),
                            )
                        o_sb = outp.tile([128, 512], f16, name="o_sb")
                        # residual add fused into the evacuation (replaces an
                        # identity matmul + copy); on DVE to keep ScalarE clear
                        # for the critical-path softmax exp
                        nc.vector.tensor_add(
                            o_sb[:], o_ps[:], z_sb[:, cc, ns * 512:(ns + 1) * 512]
                        )
                        nc.sync.dma_start(
                            dram[f"out_{br}"][s, cc * 128:(cc + 1) * 128, ns * 512:(ns + 1) * 512],
                            o_sb[:],
                        )

                    for ns in range(N // 512):
                        for cc in range(2):
                            thunks.append(
                                lambda f=tile_thunk, ns=ns, cc=cc: f(ns=ns, cc=cc)
                            )
                return thunks

            pending = []

            def drain(k):
                for _ in range(min(k, len(pending))):
                    pending.pop(0)()

            samples = [s for _ in range(outer_reps) for s in range(BPC)]
            prefetched = a0_prefetch_all(samples[0])
            for si, s in enumerate(samples):
                states = prefetched
                if si + 1 < len(samples):
                    prefetched = a0_prefetch_all(samples[si + 1])
                for br in BRANCHES:
                    a1_qk(states[br], br)
                drain(6)
                for br in BRANCHES:
                    a2_energy_softmax(states[br], br)
                drain(6)
                for br in BRANCHES:
                    a3_transpose(states[br], br)
                for br in BRANCHES:
                    a4_w2t(states[br], br)
                drain(len(pending))
                pending = b_thunks(s, states)
            drain(len(pending))

    nc.compile()
    return nc


def _numpy_fallback(inp):
    # Exact fp32 mirror of the reference; only used if conv biases are
    # nonzero (setup_inputs always produces zero biases).
    def conv(x, w, b):
        return np.einsum("bchw,oc->bohw", x, w) + b[None, :, None, None]

    def branch(rgb, z, qw, qb, kw, kb, vw, vb):
        Bb, Cc, H, W = rgb.shape
        q = conv(rgb, qw, qb).reshape(Bb, Cc, -1)
        k = conv(rgb, kw, kb).reshape(Bb, Cc, -1)
        e = np.einsum("bcm,bdm->bcd", q, k)
        e = e.max(-1, keepdims=True) - e
        e = e - e.max(-1, keepdims=True)
        a = np.exp(e)
        a /= a.sum(-1, keepdims=True)
        v = conv(z, vw, vb).reshape(Bb, Cc, -1)
        return np.einsum("bcd,bdn->bcn", a, v).reshape(Bb, Cc, H, W)

    t = inp["alpha"] * branch(inp["rgb_t_map"], inp["z_t_map"], inp["tq_w"], inp["tq_b"],
                              inp["tk_w"], inp["tk_b"], inp["tv_w"], inp["tv_b"]) + inp["z_t_map"]
    s = inp["beta"] * branch(inp["rgb_s_map"], inp["z_s_map"], inp["sq_w"], inp["sq_b"],
                             inp["sk_w"], inp["sk_b"], inp["sv_w"], inp["sv_b"]) + inp["z_s_map"]
    return (t.astype(np.float32), s.astype(np.float32))


def _run(in_maps, trace=False):
    from concourse import bass_utils

    if "nc" not in _CACHE:
        _CACHE["nc"] = _build()
    return bass_utils.run_bass_kernel_spmd(
        _CACHE["nc"], in_maps, core_ids=list(range(N_CORES)), trace=trace
    )


def _prep_in_maps(inputs):
    inp = {k: np.asarray(v) for k, v in inputs.items()}
    x_t = inp["rgb_t_map"].reshape(B, C, -1).astype(BF16)
    z_t = inp["z_t_map"].reshape(B, C, -1).astype(BF16)
    x_s = inp["rgb_s_map"].reshape(B, C, -1).astype(BF16)
    z_s = inp["z_s_map"].reshape(B, C, -1).astype(BF16)
    # attn rows/cols live in the sigma-permuted basis (col g = p*32+o holds
    # channel o*8+p), so vw's contraction rows get the same permutation.
    g = np.arange(C)
    sigma = (g % 32) * 8 + g // 32

    def split(a):  # [256, W] -> [128, 2*W] (partition p holds rows p and 128+p)
        return a.reshape(2, 128, -1).transpose(1, 0, 2).reshape(128, -1)

    consts = np.concatenate(
        [
            np.eye(128, dtype=np.float32),
            split(np.concatenate([inp["tq_w"].T, inp["tk_w"].T], axis=1)),
            split(np.concatenate([inp["sq_w"].T, inp["sk_w"].T], axis=1)),
            split((float(inp["alpha"][0]) * inp["tv_w"])[sigma, :]),
            split((float(inp["beta"][0]) * inp["sv_w"])[sigma, :]),
        ],
        axis=1,
    )
    rep = {"consts": np.ascontiguousarray(consts).astype(BF16)}
    in_maps = []
    for i in range(N_CORES):
        sl = slice(i * BPC, (i + 1) * BPC)
        in_maps.append({
            "x_t": np.ascontiguousarray(x_t[sl]),
            "z_t": np.ascontiguousarray(z_t[sl]),
            "x_s": np.ascontiguousarray(x_s[sl]),
            "z_s": np.ascontiguousarray(z_s[sl]),
            **rep,
        })
    return in_maps


def kernel(**inputs):
    inp = {k: np.asarray(v) for k, v in inputs.items()}
    if any(np.any(inp[k]) for k in ("tq_b", "tk_b", "tv_b", "sq_b", "sk_b", "sv_b")):
        return _numpy_fallback(inp)

    res = _run(_prep_in_maps(inp))
    out_t = np.concatenate([r["out_t"] for r in res.results]).reshape(B, C, 32, 32)
    out_s = np.concatenate([r["out_s"] for r in res.results]).reshape(B, C, 64, 64)
    return (out_t.astype(np.float32), out_s.astype(np.float32))
